# revision 42
# baseline (speedup 1.0000x reference)
"""Trainium2 Bass kernel for nn_DenseRelativeLoc.

Data-parallel over batch: 32 images per NeuronCore x 8 cores.

v11 pipeline design:
  * Pair 0 (batches 0-1) is computed without the SWDGE gather: the
    position selection runs on the PE as one-hot matmuls
    (feats = x^T @ E), so real work starts at ~8us instead of waiting
    ~22us for the gpsimd gather library to load.
  * Pair 1 (batches 2-3) uses per-batch gathers (first data right at
    lib-load completion), pairs 2+ per-pair gathers. Index padding is
    -1 so the SWDGE descriptor build stops at the real count.
  * DMA queues: sync carries idxt first (gathers need it), then w1
    per-chunk in consumption order, then w2/w3/biases; scalar carries
    only the 4 small one-hot operand tensors.
  * GEMM2/GEMM3 of pair P-1 are emitted between the a-half and b-half
    of pair P's GEMM1.
"""
import sys
import types

import numpy as np

B, C, H, W_IMG = 256, 768, 14, 14
S = 196          # sample count == H*W
HID = 512
OUT = 2
N_CORES = 8
BPC = B // N_CORES      # 32 batches per core
PAIRS = BPC // 2        # 16
KC = C // 128           # 6 chunks per branch
MJ = HID // 128         # 4 HID chunks
S2 = 2 * S              # 392: pair width
NIDX = 512              # padded idx count per gather (2*S=392 -> %128)
IDXW = NIDX // 16       # 32
NREAL = S2              # valid idx per gather
NIDX_B = 256            # padded idx count for the per-batch pair-1 gathers
IDXW_B = NIDX_B // 16   # 16
WARM = 64               # p-state warmup matmuls
PREF = 3                # gather prefetch depth (pairs ahead)
NOH = 2                 # batches computed via one-hot select (pair 0)


def _install_ntff_hook():
    try:
        import antenv.axon_hooks  # noqa: F401
        return
    except ImportError:
        pass
    try:
        from trn_agent_boot.trn_boot import _ntff_profile_via_ctypes
        hook = _ntff_profile_via_ctypes("/opt/axon/libaxon_pjrt.so")
    except Exception:
        hook = None
    mod = types.ModuleType("antenv.axon_hooks")
    mod.get_axon_ntff_profile_hook = lambda: hook
    sys.modules["antenv.axon_hooks"] = mod


def _build_nc():
    from contextlib import ExitStack

    import concourse.bass as bass
    import concourse.bacc as bacc
    import concourse.mybir as mybir
    import concourse.tile as tile

    dt = mybir.dt
    f32, bf16, i32, i16 = dt.float32, dt.bfloat16, dt.int32, dt.int16
    AF = mybir.ActivationFunctionType
    ALU = mybir.AluOpType

    nc = bacc.Bacc(None, target_bir_lowering=False)

    xt_t = nc.dram_tensor("xT", [BPC * S, C], bf16, kind="ExternalInput")
    w1_t = nc.dram_tensor("W1", [128, 2 * KC, HID], bf16, kind="ExternalInput")
    w2_t = nc.dram_tensor("W2", [HID, HID], bf16, kind="ExternalInput")
    w3_t = nc.dram_tensor("W3", [HID, OUT], bf16, kind="ExternalInput")
    b1_t = nc.dram_tensor("b1", [HID], f32, kind="ExternalInput")
    b2_t = nc.dram_tensor("b2", [HID], f32, kind="ExternalInput")
    b3_t = nc.dram_tensor("b3", [OUT], f32, kind="ExternalInput")
    idx_t = nc.dram_tensor("idxg", [128, 4 * IDXW_B + (PAIRS - 2) * 2 * IDXW],
                           i16, kind="ExternalInput")
    xp_t = nc.dram_tensor("xposT", [128, NOH, 2, KC, 128], bf16,
                          kind="ExternalInput")
    e_t = nc.dram_tensor("eoh", [128, NOH, 2, 2, S], bf16,
                         kind="ExternalInput")
    pxs_t = nc.dram_tensor("pxs", [BPC, S2], i32, kind="ExternalInput")
    pys_t = nc.dram_tensor("pys", [BPC, S2], i32, kind="ExternalInput")
    pred_t = nc.dram_tensor("predT", [OUT, BPC * S], f32, kind="ExternalOutput")
    delta_t = nc.dram_tensor("deltaxy", [BPC * S, OUT], f32,
                             kind="ExternalOutput")

    from concourse import library_config

    with ExitStack() as ctx:
        tc = ctx.enter_context(tile.TileContext(nc))
        nc.gpsimd.load_library(library_config.mlp)
        wpool = ctx.enter_context(tc.tile_pool(name="w", bufs=1))
        gxpool = ctx.enter_context(tc.tile_pool(name="gx", bufs=PREF + 2))
        h1pool = ctx.enter_context(tc.tile_pool(name="h1", bufs=2))
        h2pool = ctx.enter_context(tc.tile_pool(name="h2", bufs=2))
        opool = ctx.enter_context(tc.tile_pool(name="op", bufs=2))
        idxpool = ctx.enter_context(tc.tile_pool(name="idx", bufs=1))
        zps = ctx.enter_context(tc.tile_pool(name="zps", bufs=1, space="PSUM"))
        hps = ctx.enter_context(tc.tile_pool(name="hps", bufs=1, space="PSUM"))
        zqps = ctx.enter_context(tc.tile_pool(name="zq", bufs=1, space="PSUM"))
        fpool = ctx.enter_context(tc.tile_pool(name="f", bufs=2))

        # ---------- DMA layout ----------
        # scalar queue: only the 4 small one-hot operands (first compute
        # consumer). sync queue: idxt first, then w1 chunks in
        # consumption order, then everything else.
        xpsb = wpool.tile([128, NOH, 2, KC, 128], bf16, name="xpsb",
                          tag="xpsb")
        esb = wpool.tile([128, NOH, 2, 2, S], bf16, name="esb", tag="esb")
        w1sb = wpool.tile([128, 2 * KC, HID], bf16, name="w1sb", tag="w1sb")
        idxt = idxpool.tile([128, 4 * IDXW_B + (PAIRS - 2) * 2 * IDXW], i16,
                            name="idxt", tag="idxt")

        for b in range(NOH):
            nc.scalar.dma_start(esb[:, b], e_t[:, b])
            nc.scalar.dma_start(xpsb[:, b], xp_t[:, b])

        nc.sync.dma_start(idxt[:], idx_t[:, :])
        for k in range(2 * KC):
            nc.sync.dma_start(w1sb[:, k, :], w1_t[:, k, :])
        w2b = []
        for k in range(MJ):
            wb = wpool.tile([128, HID], bf16, name=f"w2b{k}", tag=f"w2b{k}")
            nc.sync.dma_start(wb[:], w2_t[k * 128:(k + 1) * 128, :])
            w2b.append(wb)
        w3b = []
        for k in range(MJ):
            wb = wpool.tile([128, OUT], bf16, name=f"w3b{k}", tag=f"w3b{k}")
            nc.sync.dma_start(wb[:], w3_t[k * 128:(k + 1) * 128, :])
            w3b.append(wb)
        b1c, b2c = [], []
        for j in range(MJ):
            t1 = wpool.tile([128, 1], f32, name=f"b1c{j}", tag=f"b1c{j}")
            nc.sync.dma_start(t1[:], b1_t[j * 128:(j + 1) * 128])
            b1c.append(t1)
            t2 = wpool.tile([128, 1], f32, name=f"b2c{j}", tag=f"b2c{j}")
            nc.sync.dma_start(t2[:], b2_t[j * 128:(j + 1) * 128])
            b2c.append(t2)
        b3c = wpool.tile([OUT, 1], f32, name="b3c", tag="b3c")
        nc.sync.dma_start(b3c[:], b3_t[:])

        pxs_sb = idxpool.tile([BPC, S2], i32, name="pxs_sb", tag="pxs_sb")
        nc.sync.dma_start(pxs_sb[:], pxs_t[:, :])
        pys_sb = idxpool.tile([BPC, S2], i32, name="pys_sb", tag="pys_sb")
        nc.sync.dma_start(pys_sb[:], pys_t[:, :])

        # ---------- gathers ----------
        # batches 0-1 (pair 0) are produced on the PE via one-hot select,
        # batches 2-3 (pair 1) gather per-batch so the pipeline streams as
        # soon as the gather library finishes loading; pairs 2+ per-pair.
        gx_pair = {}     # P -> (ga, gb) [128, KC, NIDX] bf16
        g0_tiles = {}    # (b, br) -> [128, KC, *] bf16

        def emit_gather_pb(b):
            for br in (0, 1):
                g = wpool.tile([128, KC, NIDX_B], bf16,
                               name=f"g0_{b}{br}", tag=f"g0_{b}{br}")
                off = ((b - 2) * 2 + br) * IDXW_B
                nc.gpsimd.dma_gather(
                    g[:], xt_t[:, :], idxt[:, off:off + IDXW_B],
                    num_idxs=NIDX_B, num_idxs_reg=S, elem_size=C,
                    transpose=True,
                )
                g0_tiles[(b, br)] = g

        def emit_gather(P):
            if P in gx_pair or P >= PAIRS:
                return
            ga = gxpool.tile([128, KC, NIDX], bf16, name=f"gxa{P}", tag="gxa")
            gb = gxpool.tile([128, KC, NIDX], bf16, name=f"gxb{P}", tag="gxb")
            base = 4 * IDXW_B
            for g_out, br in ((ga, 0), (gb, 1)):
                off = base + ((P - 2) * 2 + br) * IDXW
                nc.gpsimd.dma_gather(
                    g_out[:], xt_t[:, :], idxt[:, off:off + IDXW],
                    num_idxs=NIDX, num_idxs_reg=NREAL, elem_size=C,
                    transpose=True,
                )
            gx_pair[P] = (ga, gb)

        for b in (2, 3):
            emit_gather_pb(b)
        for P in range(2, 2 + PREF + 1):
            emit_gather(P)

        ones_row = wpool.tile([1, 128], bf16, name="ones_row", tag="ones_row")
        nc.vector.memset(ones_row[:], 1.0)

        # PE warm-up so the p-state ramps before real matmuls
        wmt = zps.tile([128, S2], f32, name="warm", tag="zt0")
        for _ in range(WARM):
            nc.tensor.matmul(wmt[:, 0:128], ones_row[:], ones_row[:],
                             start=True, stop=True)

        # ---------- pair 0 via one-hot select on the PE ----------
        def emit_select(b):
            ncopy = 0
            for br in (0, 1):
                ft = fpool.tile([128, KC, S], bf16, name=f"f{b}_{br}",
                                tag=f"f{br}")
                for c2 in range(KC // 2):
                    zq = zqps.tile([128, 2, S], f32, name=f"zq{b}_{br}_{c2}",
                                   tag=f"zq{c2 % 2}")
                    for ci in (0, 1):
                        for pc in (0, 1):
                            nc.tensor.matmul(
                                zq[:, ci, :],
                                xpsb[:, b, pc, 2 * c2 + ci, :],
                                esb[:, b, br, pc, :],
                                start=(pc == 0), stop=(pc == 1),
                            )
                    if ncopy % 2 == 0:
                        nc.scalar.activation(ft[:, 2 * c2:2 * c2 + 2, :],
                                             zq[:], AF.Copy)
                    else:
                        nc.vector.tensor_copy(ft[:, 2 * c2:2 * c2 + 2, :],
                                              zq[:])
                    ncopy += 1
                g0_tiles[(b, br)] = ft

        # ---------- deltaxy on vector (gpsimd stays free for gathers) -----
        dsub = idxpool.tile([BPC, S2], i32, name="dsub", tag="dsub")
        nc.vector.tensor_tensor(dsub[:], pxs_sb[:], pys_sb[:], ALU.subtract)
        ddel = idxpool.tile([BPC, S2], f32, name="ddel", tag="ddel")
        nc.vector.tensor_scalar(ddel[:], dsub[:], float(H - 1), None,
                                op0=ALU.add)
        nc.sync.dma_start(bass.AP(delta_t, 0, [[S2, BPC], [1, S2]]), ddel[:])

        # ---------- per-pair compute ----------
        h1_pair = {}     # P -> [128, MJ, S2] bf16
        zts_pair = {}    # P -> list of PSUM tiles mid-accumulation

        def emit_proj_batch_half(b, half):
            # batches 0-3: per-batch GEMM1 (196-col matmuls), one branch
            P = b // 2
            if b % 2 == 0 and half == 0:
                h1_pair[P] = h1pool.tile([128, MJ, S2], bf16,
                                         name=f"h1_{P}", tag="h1")
                zts_pair[P] = [
                    zps.tile([128, S2], f32, name=f"zt{j}_{P}", tag=f"zt{j}")
                    for j in range(MJ)]
            h1 = h1_pair[P]
            zts = zts_pair[P]
            c0 = (b % 2) * S
            g = g0_tiles[(b, half)]
            for j in range(MJ):
                for k in range(KC):
                    nc.tensor.matmul(
                        zts[j][:, c0:c0 + S],
                        w1sb[:, half * KC + k, j * 128:(j + 1) * 128],
                        g[:, k, 0:S],
                        start=(half == 0 and k == 0),
                        stop=(half == 1 and k == KC - 1),
                    )
                if half == 1:
                    if j < 2:
                        nc.scalar.activation(h1[:, j, c0:c0 + S],
                                             zts[j][:, c0:c0 + S],
                                             AF.Relu, bias=b1c[j][:])
                    else:
                        nc.vector.tensor_scalar(h1[:, j, c0:c0 + S],
                                                zts[j][:, c0:c0 + S],
                                                b1c[j][:], 0.0,
                                                op0=ALU.add, op1=ALU.max)
            if b % 2 == 1 and half == 1:
                del zts_pair[P]

        def emit_proj_batch(b):
            emit_proj_batch_half(b, 0)
            emit_proj_batch_half(b, 1)

        def emit_proj_half(P, half):
            G = gx_pair[P]
            gx = G[half]
            if half == 0:
                h1 = h1pool.tile([128, MJ, S2], bf16, name=f"h1_{P}", tag="h1")
                h1_pair[P] = h1
                zts_pair[P] = [
                    zps.tile([128, S2], f32, name=f"zt{j}_{P}", tag=f"zt{j}")
                    for j in range(MJ)]
            h1 = h1_pair[P]
            zts = zts_pair[P]
            for j in range(MJ):
                for k in range(KC):
                    nc.tensor.matmul(
                        zts[j][:],
                        w1sb[:, half * KC + k, j * 128:(j + 1) * 128],
                        gx[:, k, 0:S2],
                        start=(half == 0 and k == 0),
                        stop=(half == 1 and k == KC - 1),
                    )
                if half == 1:
                    if j < 2:
                        nc.scalar.activation(h1[:, j, :], zts[j][:],
                                             AF.Relu, bias=b1c[j][:])
                    else:
                        nc.vector.tensor_scalar(h1[:, j, :], zts[j][:],
                                                b1c[j][:], 0.0,
                                                op0=ALU.add, op1=ALU.max)
            if half == 1:
                del zts_pair[P]

        def emit_tail(P):
            h1 = h1_pair.pop(P)
            h2 = h2pool.tile([128, MJ, S2], bf16, name=f"h2_{P}", tag="h2")
            for j in range(MJ):
                hp = hps.tile([128, S2], f32, name=f"h2ps{j}_{P}",
                              tag=f"hps{j % 2}")
                for k in range(MJ):
                    nc.tensor.matmul(
                        hp[:],
                        w2b[k][:, j * 128:(j + 1) * 128],
                        h1[:, k, :],
                        start=(k == 0), stop=(k == MJ - 1),
                    )
                if j < 2:
                    nc.scalar.activation(h2[:, j, :], hp[:], AF.Relu,
                                         bias=b2c[j][:])
                else:
                    nc.vector.tensor_scalar(h2[:, j, :], hp[:], b2c[j][:],
                                            0.0, op0=ALU.add, op1=ALU.max)
            pp = hps.tile([OUT, S2], f32, name=f"pps_{P}", tag="hps0")
            po = opool.tile([OUT, S2], f32, name=f"po_{P}", tag="po")
            if P == PAIRS - 1:
                # final pair: drain in halves so the last output DMA
                # starts as early as possible
                for h in (0, 1):
                    cs = slice(h * S, h * S + S)
                    for k in range(MJ):
                        nc.tensor.matmul(pp[:, cs], w3b[k][:], h2[:, k, cs],
                                         start=(k == 0), stop=(k == MJ - 1))
                    nc.vector.tensor_scalar(po[:, cs], pp[:, cs], b3c[:],
                                            None, op0=ALU.add)
                    nc.sync.dma_start(
                        pred_t[:, P * S2 + h * S:P * S2 + h * S + S],
                        po[:, cs])
            else:
                for k in range(MJ):
                    nc.tensor.matmul(pp[:], w3b[k][:], h2[:, k, :],
                                     start=(k == 0), stop=(k == MJ - 1))
                nc.vector.tensor_scalar(po[:], pp[:], b3c[:], None,
                                        op0=ALU.add)
                nc.sync.dma_start(pred_t[:, P * S2:(P + 1) * S2], po[:])

        # ---------- main loop ----------
        emit_select(0)
        emit_proj_batch(0)
        emit_select(1)
        emit_proj_batch(1)
        emit_proj_batch(2)
        emit_tail(0)
        emit_proj_batch(3)
        for P in range(2, PAIRS):
            emit_gather(P + PREF + 1)
            emit_proj_half(P, 0)
            emit_tail(P - 1)
            emit_proj_half(P, 1)
            gx_pair.pop(P, None)
        emit_tail(PAIRS - 1)

    nc.finalize()
    return nc


_NC = None


def _get_nc():
    global _NC
    if _NC is None:
        _install_ntff_hook()
        _NC = _build_nc()
    return _NC


def _make_in_maps(inputs):
    import ml_dtypes
    bf16 = ml_dtypes.bfloat16

    x = np.asarray(inputs["x"], dtype=np.float32).reshape(B, C, H * W_IMG)
    x = np.asarray(x, dtype=bf16)

    W1 = np.asarray(np.asarray(inputs["W1"], dtype=np.float32), dtype=bf16)
    # [2C, HID] -> [128, 12, HID]
    w1p = np.ascontiguousarray(
        W1.reshape(2, KC, 128, HID).transpose(2, 0, 1, 3)
    ).reshape(128, 2 * KC, HID)

    W2 = np.asarray(np.asarray(inputs["W2"], dtype=np.float32), dtype=bf16)
    W3 = np.asarray(np.asarray(inputs["W3"], dtype=np.float32), dtype=bf16)
    b1 = np.asarray(inputs["b1"], dtype=np.float32)
    b2 = np.asarray(inputs["b2"], dtype=np.float32)
    b3 = np.asarray(inputs["b3"], dtype=np.float32)
    pxs = np.asarray(inputs["pxs"], dtype=np.int32)
    pys = np.asarray(inputs["pys"], dtype=np.int32)
    idx_x = pxs[:, :, 0] * W_IMG + pxs[:, :, 1]     # [B, S]
    idx_y = pys[:, :, 0] * W_IMG + pys[:, :, 1]

    in_maps = []
    for c in range(N_CORES):
        sl = slice(c * BPC, (c + 1) * BPC)
        # xT [BPC*S, C]
        xT = np.ascontiguousarray(
            x[sl].transpose(0, 2, 1).reshape(BPC * S, C))

        # global row indices; trailing pad = -1 so the SWDGE descriptor
        # build stops at the real count. Batches 2-3 get per-batch index
        # lists (NIDX_B each), pairs 2+ get per-pair lists (NIDX each).
        lix, liy = idx_x[sl], idx_y[sl]

        def wrap16(vals, nidx):
            a = np.full(nidx, -1, dtype=np.int16)
            a[:len(vals)] = vals
            return a.reshape(nidx // 16, 16).T       # [16, nidx//16]

        cols = []
        for b in (2, 3):
            for idx in (lix, liy):
                cols.append(wrap16(idx[b] + b * S, NIDX_B))
        for P in range(2, PAIRS):
            for idx in (lix, liy):
                vals = np.concatenate([idx[2 * P] + 2 * P * S,
                                       idx[2 * P + 1] + (2 * P + 1) * S])
                cols.append(wrap16(vals, NIDX))
        idx16 = np.concatenate(cols, axis=1).astype(np.int16)
        assert idx16.shape[1] == 4 * IDXW_B + (PAIRS - 2) * 2 * IDXW
        idxw = np.ascontiguousarray(np.tile(idx16, (8, 1)))

        # one-hot select operands for batches 0-1 (pair 0):
        # xposT[p, b, pc, c, :] = x[b].T rows pc*128+p, channels c*128:+128
        xpos = np.zeros((128, NOH, 2, KC, 128), dtype=bf16)
        eoh = np.zeros((128, NOH, 2, 2, S), dtype=bf16)
        for b in range(NOH):
            xbT = np.zeros((256, C), dtype=bf16)
            xbT[:S] = x[sl][b].T                     # [196, 768] bf16
            xpos[:, b] = xbT.reshape(2, 128, KC, 128).transpose(1, 0, 2, 3)
            for br, idx in ((0, lix), (1, liy)):
                pos = idx[b]                         # [S] in [0, 196)
                eoh[pos % 128, b, br, pos // 128, np.arange(S)] = 1.0

        in_maps.append({
            "xT": xT,
            "W1": w1p, "W2": W2, "W3": W3,
            "b1": b1, "b2": b2, "b3": b3,
            "idxg": idxw,
            "xposT": xpos, "eoh": eoh,
            "pxs": np.ascontiguousarray(pxs[sl].reshape(BPC, S2)),
            "pys": np.ascontiguousarray(pys[sl].reshape(BPC, S2)),
        })
    return in_maps


def _run(inputs, trace=False):
    from concourse.bass_utils import run_bass_kernel_spmd

    nc = _get_nc()
    in_maps = _make_in_maps(inputs)
    res = run_bass_kernel_spmd(
        nc, in_maps, core_ids=list(range(N_CORES)), trace=trace
    )
    pred = np.concatenate(
        [np.ascontiguousarray(res.results[c]["predT"].T) for c in range(N_CORES)],
        axis=0,
    )
    delta = np.concatenate(
        [res.results[c]["deltaxy"] for c in range(N_CORES)], axis=0
    )
    return (pred, delta), res


def kernel(**inputs):
    (pred, delta), _ = _run(inputs, trace=False)
    return pred, delta
